# revision 1
# baseline (speedup 1.0000x reference)
"""Trainium2 Bass kernel for causal multi-head attention (B=2, S=2048, D=1024, 16 heads x 64).

Sharding: 8 cores = 2 batches x 4 head-groups (tensor parallel over heads).
Each core computes attention for its 4 heads; the 4 cores of a batch then
AllToAll-exchange normalized head outputs by q-quarter (2MB instead of an
8.4MB ReduceScatter on the projected output), and each core applies the full
W_O projection locally to its 512-row shard. Host concatenates the shards.

Attention is flash-style with transposed scores:
  sT[k, q] = K Q^T  (k on partitions), pattern = exp(sT/8) on ACT,
  AV uses stationary [v | 1] so PSUM row 64 accumulates the softmax
  denominator for free. Causality at tile granularity: upper-triangular
  tiles are skipped and diagonal tiles are column-trimmed + masked with a
  single 128x128 triangle.
All matmuls run in fp32r (full-rate reduced-precision fp32); every matmul
input is produced by an fp32r-rounding instruction chain (DMA of fp32r
tensors or engine writes to fp32r tiles).
"""

import os
import sys

sys.path.insert(0, "/opt/trn_rl_repo")

import numpy as np

# ---- problem constants (hardcoded; kernel.py must be self-contained) ----
B = 2
S = 2048
D = 1024
N_HEADS = 16
DH = 64                 # head dim
NCORES = 8
NH_CORE = N_HEADS // 4  # 4 heads per core (4-way TP x 2-way batch DP)
SCALE = 1.0 / 8.0       # 1/sqrt(64)

P = 128                 # partitions
DC = D // P             # 8 contraction chunks for the projections
KC = S // P             # 16 key chunks
QT = 512                # q tile width (free dim) for scores / AV
NQT = S // QT           # 4 q tiles
NT = 512                # moving-operand tile for projections / out-proj
GRP = 4                 # cores per batch group

_CACHE = {}


def _build():
    import concourse.bass as bass
    import concourse.tile as tile
    from concourse import bacc, mybir

    f32 = mybir.dt.float32
    MM = mybir.dt.float32r
    F16 = mybir.dt.float16

    nc = bacc.Bacc(
        "TRN2",
        target_bir_lowering=False,
        debug=False,
        enable_asserts=False,
        num_devices=NCORES,
    )

    xt_d = nc.dram_tensor("xt", [D, S], F16, kind="ExternalInput").ap()
    wqt_d = nc.dram_tensor("wqt", [D, NH_CORE * DH], F16, kind="ExternalInput").ap()
    wkt_d = nc.dram_tensor("wkt", [D, NH_CORE * DH], F16, kind="ExternalInput").ap()
    wvt_d = nc.dram_tensor("wvt", [D, NH_CORE * DH], F16, kind="ExternalInput").ap()
    wof_d = nc.dram_tensor("wof", [N_HEADS * DH, D], F16, kind="ExternalInput").ap()
    msk_d = nc.dram_tensor("msk", [P, P], F16, kind="ExternalInput").ap()
    out_d = nc.dram_tensor("out", [S // GRP, D], f32, kind="ExternalOutput").ap()
    dbg_mode = int(os.environ.get("KERNEL_DEBUG", "0"))
    dbg_d = None
    if dbg_mode == 1:
        dbg_d = nc.dram_tensor("dbg", [NQT * 2 * P, QT], f32, kind="ExternalOutput").ap()
    elif dbg_mode == 2:
        dbg_d = nc.dram_tensor("dbg", [NQT * NCORES * 2 * P, QT], f32, kind="ExternalOutput").ap()

    Exp = mybir.ActivationFunctionType.Exp

    with tile.TileContext(nc) as tc:
        with (
            tc.tile_pool(name="const", bufs=1) as const,
            tc.tile_pool(name="work", bufs=2) as work,
            tc.tile_pool(name="ps", bufs=2, space="PSUM") as ps_pool,
            tc.tile_pool(name="dram", bufs=1, space="DRAM") as dram,
        ):
            late_cm = tc.tile_pool(name="late", bufs=1)
            late = late_cm.__enter__()
            xt_pool_cm = tc.tile_pool(name="xtp", bufs=1)
            xt_pool = xt_pool_cm.__enter__()
            # ---------------- input DMAs ----------------
            wq_sb = xt_pool.tile([P, DC, NH_CORE * DH], F16)
            nc.sync.dma_start(wq_sb[:], wqt_d.rearrange("(c p) n -> p c n", p=P))

            # residual^T, split per d-chunk so QKV matmuls can start early
            xt_sb = xt_pool.tile([P, DC, S], F16)
            xt_r = xt_d.rearrange("(c p) s -> p c s", p=P)
            for dc in range(DC):
                nc.sync.dma_start(xt_sb[:, dc, :], xt_r[:, dc, :])

            wk_sb = xt_pool.tile([P, DC, NH_CORE * DH], F16)
            nc.sync.dma_start(wk_sb[:], wkt_d.rearrange("(c p) n -> p c n", p=P))
            wv_sb = xt_pool.tile([P, DC, NH_CORE * DH], F16)
            nc.sync.dma_start(wv_sb[:], wvt_d.rearrange("(c p) n -> p c n", p=P))
            tri_sb = const.tile([P, P], F16)
            nc.sync.dma_start(tri_sb[:], msk_d)

            # ---------------- QKV projections ----------------
            qT = [const.tile([P, S], F16, name=f"qT{i}") for i in range(2)]
            kT = [const.tile([P, S], F16, name=f"kT{i}") for i in range(2)]

            def qk_proj(ntile):
                for pr in range(2):
                    for w_sb, dst in ((wq_sb, qT[pr]), (wk_sb, kT[pr])):
                        pp = ps_pool.tile(
                            [P, NT], f32, name="pp", tag=f"s0{ntile % 2}", bufs=1
                        )
                        for dc in range(DC):
                            nc.tensor.matmul(
                                pp[:],
                                w_sb[:, dc, pr * P : (pr + 1) * P],
                                xt_sb[:, dc, ntile * NT : (ntile + 1) * NT],
                                start=(dc == 0),
                                stop=(dc == DC - 1),
                            )
                        nc.scalar.copy(dst[:, ntile * NT : (ntile + 1) * NT], pp[:])
            for nt in range(2):
                qk_proj(nt)

            # v in natural [k, h] layout with an appended ones column
            v_aug = [const.tile([P, KC, DH + 1], F16, name=f"vaug{h}") for h in range(NH_CORE)]
            ones_f32 = const.tile([P, DH], f32)
            nc.vector.memset(ones_f32[:], 1.0)
            ones_sb = const.tile([P, DH], F16)
            nc.scalar.copy(ones_sb[:], ones_f32[:])
            for h in range(NH_CORE):
                nc.scalar.copy(v_aug[h][:, :, DH : DH + 1], ones_f32[:, 0:KC, None])
            def v_proj(pc):
                vp = ps_pool.tile(
                    [P, NH_CORE * DH], f32, name="vp", tag=f"s1{pc % 2}", bufs=1
                )
                for dc in range(DC):
                    nc.tensor.matmul(
                        vp[:],
                        xt_sb[:, dc, pc * P : (pc + 1) * P],
                        wv_sb[:, dc, :],
                        start=(dc == 0),
                        stop=(dc == DC - 1),
                    )
                for h in range(NH_CORE):
                    nc.vector.tensor_copy(
                        v_aug[h][:, pc, 0:DH],
                        vp[:, h * DH : (h + 1) * DH],
                    )

            # cc_in rows [pr*128 + h2*64 : +64] = normalized attn_outT of head
            # (pr*2+h2), all q. AllGather runs per q-quarter column stripe,
            # one quarter behind the attention loop so it overlaps compute.
            # cc_out rows [i*256:(i+1)*256] = core i's heads (global head
            # order matches W_O rows).
            cc_in = dram.tile([NQT * 2 * P, QT], F16)
            cc_out = dram.tile([NQT * NCORES * 2 * P, QT], F16)

            def norm_and_gather(qtg):
                g_sl = slice(qtg * QT, (qtg + 1) * QT)
                for pr in range(2):
                    for h2 in range(2):
                        rb_ps = ps_pool.tile(
                            [DH, QT], f32, name="rb", tag=f"acc{pr}{h2}", bufs=1
                        )
                        nc.tensor.matmul(
                            rb_ps[:],
                            ones_f32[DH : DH + 1, :],
                            uraw[pr][h2][DH : DH + 1, g_sl],
                            start=True,
                            stop=True,
                        )
                        u_n = work.tile([DH, QT], F16, name="u_n", bufs=4)
                        nc.vector.tensor_mul(
                            u_n[:], uraw[pr][h2][0:DH, g_sl], rb_ps[:]
                        )
                        row = qtg * 2 * P + pr * P + h2 * DH
                        nc.sync.dma_start(cc_in[row : row + DH, :], u_n[:])
                nc.gpsimd.collective_compute(
                    "AllGather",
                    mybir.AluOpType.bypass,
                    replica_groups=[[0, 1, 2, 3, 4, 5, 6, 7]],
                    ins=[cc_in[qtg * 2 * P : (qtg + 1) * 2 * P, :].opt()],
                    outs=[
                        cc_out[
                            qtg * NCORES * 2 * P : (qtg + 1) * NCORES * 2 * P, :
                        ].opt()
                    ],
                )

            # ---------------- attention ----------------
            # unnormalized attn_outT + denominator, per (pair, head): [65, S]
            uraw = [
                [late.tile([DH + 1, S], f32, name=f"uraw{pr}{h2}") for h2 in range(2)]
                for pr in range(2)
            ]
            for pc in range(DC):
                v_proj(pc)
            qt_order = [1, 2, 3, 0]  # dense-ish start, early AG pipeline, small tail
            for qi, qt in enumerate(qt_order):
                if qi == 1:
                    # rest of the projections, overlapping qt1's epilogue
                    for nt in range(2, S // NT):
                        qk_proj(nt)
                    for pc in range(DC, KC):
                        v_proj(pc)
                    xt_pool_cm.__exit__(None, None, None)  # frees xt + wq/wk/wv
                    wo_cm = tc.tile_pool(name="wop", bufs=1)
                    wop = wo_cm.__enter__()
                    # full W_O (needed only after the collective; DMA it late)
                    wo_sb = wop.tile([P, DC, D], F16)
                    nc.sync.dma_start(
                        wo_sb[:], wof_d.rearrange("(c p) d -> p c d", p=P)
                    )
                q_sl = slice(qt * QT, (qt + 1) * QT)
                nk = (qt + 1) * (QT // P)
                attn_ps = [
                    [
                        ps_pool.tile([P, QT], f32, name=f"attn{pr}{h2}", tag=f"acc{pr}{h2}", bufs=1)
                        for h2 in range(2)
                    ]
                    for pr in range(2)
                ]
                for kb in range(nk):
                    k_sl = slice(kb * P, (kb + 1) * P)
                    ri = kb - qt * (QT // P)  # >= 0 on diagonal tiles
                    r = max(ri, 0) * P        # first valid column in this q tile
                    c_sl = slice(qt * QT + r, (qt + 1) * QT)
                    s_ps = [
                        [
                            ps_pool.tile([P, QT], f32, name=f"s{pr}{h2}", tag=f"s{pr}{h2}", bufs=1)
                            for h2 in range(2)
                        ]
                        for pr in range(2)
                    ]
                    for pr in range(2):
                        for h2 in range(2):
                            hb = h2 * DH
                            nc.tensor.matmul(
                                s_ps[pr][h2][:, r:QT],
                                kT[pr][hb : hb + DH, k_sl],
                                qT[pr][hb : hb + DH, c_sl],
                                start=True,
                                stop=True,
                            )
                    for pr in range(2):
                        for h2 in range(2):
                            h = pr * 2 + h2
                            pat = work.tile([P, QT], F16, name="pat", bufs=4)
                            nc.scalar.activation(
                                pat[:, r:QT], s_ps[pr][h2][:, r:QT], Exp, scale=SCALE
                            )
                            if ri >= 0:
                                nc.vector.tensor_mul(
                                    pat[:, r : r + P], pat[:, r : r + P], tri_sb[:]
                                )
                            nc.tensor.matmul(
                                attn_ps[pr][h2][0 : DH + 1, r:QT],
                                v_aug[h][:, kb, :],
                                pat[:, r:QT],
                                start=(kb == 0),
                                stop=(kb == nk - 1),
                            )
                for pr in range(2):
                    for h2 in range(2):
                        # evacuate accumulator quickly (frees PSUM for next qt)
                        nc.vector.tensor_copy(uraw[pr][h2][:, q_sl], attn_ps[pr][h2][0 : DH + 1, :])
                if qi > 0:
                    norm_and_gather(qt_order[qi - 1])
                for pr in range(2):
                    for h2 in range(2):
                        nc.vector.reciprocal(
                            uraw[pr][h2][DH : DH + 1, q_sl],
                            uraw[pr][h2][DH : DH + 1, q_sl],
                        )
            norm_and_gather(qt_order[-1])

            if dbg_mode == 1:
                pass
            elif dbg_mode == 2:
                pass
            # ------------- local W_O projection on own q-quarter -------------
            # q-quarter = group rank = partition_id % 4 (dynamic column slice)
            # my stripe: rows [(pid%4)*2048 + (pid//4)*1024 ...] in 128-chunks
            pid = nc.partition_id()
            aoff = nc.snap(
                nc.s_assert_within(
                    (pid % GRP) * 16 + (pid - (pid % GRP)) * 2,
                    0,
                    NQT * NCORES * 2 - DC,
                    skip_runtime_assert=True,
                )
            )
            attR = wop.tile([P, DC, QT], F16)
            cc_r = cc_out[:].rearrange("(a p) q -> p a q", p=P)
            nc.sync.dma_start(attR[:], cc_r[:, bass.ds(aoff, DC), :])
            for pc in range(QT // P):
                osb = work.tile([P, D], f32, name="osb", bufs=2)
                for dt_ in range(D // NT):
                    op = ps_pool.tile([P, NT], f32, name="op", tag=f"s0{dt_ % 2}", bufs=1)
                    for c in range(DC):
                        nc.tensor.matmul(
                            op[:],
                            attR[:, c, pc * P : (pc + 1) * P],
                            wo_sb[:, c, dt_ * NT : (dt_ + 1) * NT],
                            start=(c == 0),
                            stop=(c == DC - 1),
                        )
                    nc.scalar.copy(osb[:, dt_ * NT : (dt_ + 1) * NT], op[:])
                nc.sync.dma_start(out_d[pc * P : (pc + 1) * P, :], osb[:])
            wo_cm.__exit__(None, None, None)
            late_cm.__exit__(None, None, None)

    nc.compile()
    return nc


def _get_nc():
    if "nc" not in _CACHE:
        _CACHE["nc"] = _build()
    return _CACHE["nc"]


def _tri():
    k = np.arange(P)[:, None]
    q = np.arange(P)[None, :]
    return (q >= k).astype(np.float32)


def _ensure_ntff_hook():
    """Register the axon NTFF profile hook (missing antenv.axon_hooks shim)."""
    import types

    try:
        from antenv.axon_hooks import get_axon_ntff_profile_hook  # noqa: F401

        return
    except ImportError:
        pass
    import antenv

    if "/root/.axon_site" not in sys.path:
        sys.path.insert(0, "/root/.axon_site")
    from trn_agent_boot.trn_boot import _ntff_profile_via_ctypes

    hook = _ntff_profile_via_ctypes("/opt/axon/libaxon_pjrt.so")
    mod = types.ModuleType("antenv.axon_hooks")
    mod.get_axon_ntff_profile_hook = lambda: hook
    mod.set_axon_ntff_profile_hook = lambda h: None
    sys.modules["antenv.axon_hooks"] = mod
    antenv.axon_hooks = mod


def kernel(residual, W_Q, W_K, W_V, W_O):
    from concourse.bass_utils import run_bass_kernel_spmd

    if int(os.environ.get("KERNEL_TRACE", "0")):
        _ensure_ntff_hook()

    residual = np.ascontiguousarray(np.asarray(residual), np.float32)
    W_Q = np.ascontiguousarray(np.asarray(W_Q), np.float32)
    W_K = np.ascontiguousarray(np.asarray(W_K), np.float32)
    W_V = np.ascontiguousarray(np.asarray(W_V), np.float32)
    W_O = np.ascontiguousarray(np.asarray(W_O), np.float32)

    nc = _get_nc()
    tri = _tri()
    wof = np.ascontiguousarray(W_O.reshape(N_HEADS * DH, D).astype(np.float16))
    in_maps = []
    for c in range(NCORES):
        b, g = divmod(c, GRP)
        hs = slice(g * NH_CORE, (g + 1) * NH_CORE)
        in_maps.append(
            {
                "xt": np.ascontiguousarray(residual[b].T.astype(np.float16)),
                "wqt": np.ascontiguousarray(
                    W_Q[hs].transpose(2, 0, 1).reshape(D, NH_CORE * DH).astype(np.float16)
                ),
                "wkt": np.ascontiguousarray(
                    W_K[hs].transpose(2, 0, 1).reshape(D, NH_CORE * DH).astype(np.float16)
                ),
                "wvt": np.ascontiguousarray(
                    W_V[hs].transpose(2, 0, 1).reshape(D, NH_CORE * DH).astype(np.float16)
                ),
                "wof": wof,
                "msk": tri.astype(np.float16),
            }
        )

    res = run_bass_kernel_spmd(
        nc,
        in_maps,
        core_ids=list(range(NCORES)),
        trace=bool(int(os.environ.get("KERNEL_TRACE", "0"))),
        trace_cores=[0] if int(os.environ.get("KERNEL_TRACE", "0")) else None,
    )
    _CACHE["last_results"] = res

    out = np.empty((B, S, D), np.float32)
    for b in range(B):
        out[b] = np.concatenate(
            [res.results[b * GRP + r]["out"] for r in range(GRP)], axis=0
        )
    return out



# revision 10
# speedup vs baseline: 1.5762x; 1.5762x over previous
"""Trainium2 Bass kernel for causal multi-head attention (B=2, S=2048, D=1024, 16 heads x 64).

Sharding: 8 cores = 2 batches x 4 head-groups (tensor parallel over heads).
Each core computes attention for its 4 heads over the full sequence; the 4
cores of a batch AllGather normalized head outputs per q-quarter (4-rank
groups), and each core applies the full W_O projection to one 128-row block
of every quarter (so only the final AllGather gates a small slice of work).
Host concatenates the interleaved blocks.

Attention is flash-style with transposed scores: sT[k, q] = K Q^T (keys on
partitions). Both heads of a pair write one [128, 1024] 2-bank PSUM tile so a
single ACT exp covers them. AV uses stationary [v | 1] so PSUM row 64
accumulates the softmax denominator for free; the denominator row is
broadcast to 64 partitions by a 1-contraction fp32r matmul and reciprocated
at base partition 0 with the fast custom DVE op (nonzero-base custom DVE
silently no-ops). Head-pairs (pr) run sequentially per q-tile so attention
needs only 2 accumulator banks. QKV projection and out-projection matmul
chains are chopped into single-instruction units and interleaved between
attention rounds so the PE FIFO always has dependency-free filler during
ACT-bound stretches (keeps HAM warm, overlaps phases).
"""

import os
import sys

sys.path.insert(0, "/opt/trn_rl_repo")

import numpy as np

# ---- problem constants (hardcoded; kernel.py must be self-contained) ----
B = 2
S = 2048
D = 1024
N_HEADS = 16
DH = 64                 # head dim
NCORES = 8
NH_CORE = N_HEADS // 4  # 4 heads per core (4-way TP x 2-way batch DP)
SCALE = 1.0 / 8.0       # 1/sqrt(64)

P = 128                 # partitions
DC = D // P             # 8 contraction chunks for the projections
KC = S // P             # 16 key chunks
QT = 512                # q tile width (free dim) per quarter
NQT = S // QT           # 4 q tiles
GRP = 4                 # cores per batch group

_CACHE = {}


def _build():
    import concourse.bass as bass
    import concourse.tile as tile
    from concourse import bacc, mybir

    f32 = mybir.dt.float32
    f32r = mybir.dt.float32r
    F16 = mybir.dt.float16

    nc = bacc.Bacc(
        "TRN2",
        target_bir_lowering=False,
        debug=False,
        enable_asserts=False,
        num_devices=NCORES,
    )

    xt_d = nc.dram_tensor("xt", [D, S], F16, kind="ExternalInput").ap()
    wqt_d = nc.dram_tensor("wqt", [D, NH_CORE * DH], F16, kind="ExternalInput").ap()
    wkt_d = nc.dram_tensor("wkt", [D, NH_CORE * DH], F16, kind="ExternalInput").ap()
    wvt_d = nc.dram_tensor("wvt", [D, NH_CORE * DH], F16, kind="ExternalInput").ap()
    wof_d = nc.dram_tensor("wof", [N_HEADS * DH, D], F16, kind="ExternalInput").ap()
    msk_d = nc.dram_tensor("msk", [P, P], F16, kind="ExternalInput").ap()
    # 4 blocks of 128 rows: block q = rows [q*512 + g*128 .. +128] of this
    # core's batch output (g = group rank)
    out_d = nc.dram_tensor("out", [NQT * P, D], F16, kind="ExternalOutput").ap()
    dbg_d = None
    if int(os.environ.get("KERNEL_DEBUG", "0")):
        dbg_d = nc.dram_tensor("dbg", [4 * (DH + 1), S], f32, kind="ExternalOutput").ap()
    exp_pair = int(os.environ.get("KERNEL_EXPPAIR", "1"))

    Exp = mybir.ActivationFunctionType.Exp

    with tile.TileContext(nc) as tc:
        with (
            tc.tile_pool(name="const", bufs=1) as const,
            tc.tile_pool(name="work", bufs=2) as work,
            tc.tile_pool(name="ps", bufs=1, space="PSUM") as ps_pool,
            tc.tile_pool(name="dram", bufs=1, space="DRAM") as dram,
        ):
            # ---------------- input DMAs ----------------
            wq_sb = const.tile([P, DC, NH_CORE * DH], F16)
            nc.sync.dma_start(wq_sb[:], wqt_d.rearrange("(c p) n -> p c n", p=P))
            wk_sb = const.tile([P, DC, NH_CORE * DH], F16)
            nc.sync.dma_start(wk_sb[:], wkt_d.rearrange("(c p) n -> p c n", p=P))

            # residual^T: first quarter per d-chunk (small, on gpsimd queue so
            # issue overlaps the sync-engine weight DMAs), rest in fat slices
            xt_eng = nc.gpsimd if int(os.environ.get("KERNEL_XTGP", "1")) else nc.sync
            xt_sb = const.tile([P, DC, S], F16)
            xt_r = xt_d.rearrange("(c p) s -> p c s", p=P)
            for dc in range(DC):
                xt_eng.dma_start(xt_sb[:, dc, 0:QT], xt_r[:, dc, 0:QT])
            tri_sb = const.tile([P, P], F16)
            nc.sync.dma_start(tri_sb[:], msk_d)
            wv_sb = const.tile([P, DC, NH_CORE * DH], F16)
            nc.sync.dma_start(wv_sb[:], wvt_d.rearrange("(c p) n -> p c n", p=P))
            for dc in range(DC):
                xt_eng.dma_start(xt_sb[:, dc, QT:S], xt_r[:, dc, QT:S])

            # ---------------- SBUF state ----------------
            qT = [const.tile([P, S], F16, name=f"qT{i}") for i in range(2)]
            kT = [const.tile([P, S], F16, name=f"kT{i}") for i in range(2)]
            v_aug = [const.tile([P, KC, DH + 1], F16, name=f"vaug{h}") for h in range(NH_CORE)]
            ones_f32 = const.tile([P, DH], f32)
            nc.vector.memset(ones_f32[:], 1.0)
            for h in range(NH_CORE):
                nc.scalar.copy(v_aug[h][:, :, DH : DH + 1], ones_f32[:, 0:KC, None])

            # unnormalized attn_outT + denominator, per (pair, head): [65, S]
            uraw = [
                [const.tile([DH + 1, S], f32, name=f"uraw{pr}{h2}") for h2 in range(2)]
                for pr in range(2)
            ]

            wo_sb = const.tile([P, DC, D], F16)

            cc_in = dram.tile([NQT * 2 * P, QT], F16)
            cc_out = dram.tile([NQT * GRP * 2 * P, QT], F16)

            pj = [0]  # alternating tag counter for the 2 shared psum banks

            def _pj_tile(shape, name):
                t = ps_pool.tile(shape, f32, name=name, tag=f"pj{pj[0] % 2}", bufs=1)
                pj[0] += 1
                return t

            # ---- filler units: single instructions emitted between rounds ----
            def qk_chain_units(nt, pr, w_sb, dst):
                st = {}
                us = []
                for dc in range(DC):
                    def mm(dc=dc, nt=nt, pr=pr, w_sb=w_sb):
                        if dc == 0:
                            st["pp"] = _pj_tile([P, QT], "pp")
                        nc.tensor.matmul(
                            st["pp"][:],
                            w_sb[:, dc, pr * P : (pr + 1) * P],
                            xt_sb[:, dc, nt * QT : (nt + 1) * QT],
                            start=(dc == 0),
                            stop=(dc == DC - 1),
                        )
                    us.append(mm)
                def cp(nt=nt, dst=dst):
                    nc.vector.tensor_copy(dst[:, nt * QT : (nt + 1) * QT], st["pp"][:])
                us.append(cp)
                return us

            def v_chain_units(pc):
                st = {}
                us = []
                for dc in range(DC):
                    def mm(dc=dc, pc=pc):
                        if dc == 0:
                            st["vp"] = _pj_tile([P, NH_CORE * DH], "vp")
                        nc.tensor.matmul(
                            st["vp"][:],
                            xt_sb[:, dc, pc * P : (pc + 1) * P],
                            wv_sb[:, dc, :],
                            start=(dc == 0),
                            stop=(dc == DC - 1),
                        )
                    us.append(mm)
                for h in range(NH_CORE):
                    def cp(h=h, pc=pc):
                        nc.vector.tensor_copy(
                            v_aug[h][:, pc, 0:DH], st["vp"][:, h * DH : (h + 1) * DH]
                        )
                    us.append(cp)
                return us

            def proj_units(nt):
                us = []
                for pr in range(2):
                    us += qk_chain_units(nt, pr, wq_sb, qT[pr])
                    us += qk_chain_units(nt, pr, wk_sb, kT[pr])
                for pc in range(4 * nt, 4 * nt + 4):
                    us += v_chain_units(pc)
                return us

            # my 128-q-column block within each quarter (dynamic by rank)
            pid = nc.partition_id()
            qoff = nc.snap(
                nc.s_assert_within(
                    (pid % GRP) * P, 0, QT - P, skip_runtime_assert=True
                )
            )
            cc_q = cc_out[:].rearrange("(t a p) q -> t p a q", p=P, a=GRP * 2)

            def out_proj_units(qt):
                st = {}
                us = []
                def dma(qt=qt):
                    st["attR"] = work.tile([P, GRP * 2, P], F16, name="attR", bufs=2)
                    nc.sync.dma_start(
                        st["attR"][:], cc_q[qt, :, :, bass.ds(qoff, P)]
                    )
                    st["osb"] = work.tile([P, D], F16, name="osb", bufs=2)
                us.append(dma)
                for dt_ in range(D // QT):
                    for c in range(GRP * 2):
                        def mm(dt_=dt_, c=c):
                            if c == 0:
                                st[f"op{dt_}"] = _pj_tile([P, QT], "op")
                            nc.tensor.matmul(
                                st[f"op{dt_}"][:],
                                st["attR"][:, c, :],
                                wo_sb[:, c, dt_ * QT : (dt_ + 1) * QT],
                                start=(c == 0),
                                stop=(c == GRP * 2 - 1),
                            )
                        us.append(mm)
                    def cp(dt_=dt_):
                        nc.vector.tensor_copy(
                            st["osb"][:, dt_ * QT : (dt_ + 1) * QT], st[f"op{dt_}"][:]
                        )
                    us.append(cp)
                def outdma(qt=qt):
                    nc.sync.dma_start(out_d[qt * P : (qt + 1) * P, :], st["osb"][:])
                us.append(outdma)
                return us

            units = []

            def fill(rounds_left):
                if not units:
                    return
                n = max(1, (len(units) + rounds_left - 1) // max(rounds_left, 1))
                for _ in range(min(n, len(units))):
                    units.pop(0)()

            def flush():
                while units:
                    units.pop(0)()

            def norm_and_send(qt, pr):
                g_sl = slice(qt * QT, (qt + 1) * QT)
                for h2 in range(2):
                    u = uraw[pr][h2]
                    # broadcast raw denominator row to 64 partitions, then
                    # fast-reciprocal at base partition 0 (the custom DVE op
                    # silently no-ops at nonzero base partitions)
                    rb_ps = _pj_tile([DH, QT], "rb")
                    nc.tensor.matmul(
                        rb_ps[:],
                        ones_f32[DH : DH + 1, :],
                        u[DH : DH + 1, g_sl],
                        start=True,
                        stop=True,
                    )
                    rb_sb = work.tile([DH, QT], f32, name="rb_sb", bufs=2)
                    nc.vector.reciprocal_approx_fast(rb_sb[:], rb_ps[:])
                    u_n = work.tile([DH, QT], F16, name="u_n", bufs=4)
                    nc.vector.tensor_mul(u_n[:], u[0:DH, g_sl], rb_sb[:])
                    row = qt * 2 * P + pr * P + h2 * DH
                    nc.sync.dma_start(cc_in[row : row + DH, :], u_n[:])

            def gather(qt):
                nc.gpsimd.collective_compute(
                    "AllGather",
                    mybir.AluOpType.bypass,
                    replica_groups=[[0, 1, 2, 3], [4, 5, 6, 7]],
                    ins=[cc_in[qt * 2 * P : (qt + 1) * 2 * P, :].opt()],
                    outs=[
                        cc_out[qt * GRP * 2 * P : (qt + 1) * GRP * 2 * P, :].opt()
                    ],
                )

            # ---------------- main loop ----------------
            for u in proj_units(0):
                u()
            for qt in range(NQT):
                if qt == 1:
                    nc.sync.dma_start(
                        wo_sb[:], wof_d.rearrange("(c p) d -> p c d", p=P)
                    )
                # filler work for this quarter's ACT-bound attention span
                if qt + 1 < NQT:
                    units.extend(proj_units(qt + 1))
                if qt == 3:
                    units.extend(out_proj_units(1))
                    units.extend(out_proj_units(2))

                q_sl = slice(qt * QT, (qt + 1) * QT)
                nk = (qt + 1) * (QT // P)
                rounds_left = 2 * nk
                for pr in range(2):
                    acc = [
                        ps_pool.tile(
                            [DH + 1, QT], f32, name=f"acc{h2}", tag=f"acc{h2}", bufs=1
                        )
                        for h2 in range(2)
                    ]
                    for kb in range(nk):
                        k_sl = slice(kb * P, (kb + 1) * P)
                        ri = kb - qt * (QT // P)  # >= 0 on diagonal tiles
                        r = max(ri, 0) * P        # first valid col in this q tile
                        c_sl = slice(qt * QT + r, (qt + 1) * QT)
                        sc = ps_pool.tile([P, 2 * QT], f32, name="sc", tag="sc", bufs=2)
                        pat = work.tile([P, 2 * QT], F16, name="pat", bufs=3)
                        for h2 in range(2):
                            hb = h2 * DH
                            nc.tensor.matmul(
                                sc[:, h2 * QT + r : (h2 + 1) * QT],
                                kT[pr][hb : hb + DH, k_sl],
                                qT[pr][hb : hb + DH, c_sl],
                                start=True,
                                stop=True,
                            )
                        # one exp covers both heads (cols 512..512+r of the
                        # diagonal rounds are stale-PSUM garbage, never read)
                        if exp_pair:
                            nc.scalar.activation(
                                pat[:, r : 2 * QT], sc[:, r : 2 * QT], Exp, scale=SCALE
                            )
                        else:
                            for h2 in range(2):
                                e_sl = slice(h2 * QT + r, (h2 + 1) * QT)
                                nc.scalar.activation(
                                    pat[:, e_sl], sc[:, e_sl], Exp, scale=SCALE
                                )
                        for h2 in range(2):
                            p_sl = slice(h2 * QT + r, (h2 + 1) * QT)
                            if ri >= 0:
                                nc.vector.tensor_mul(
                                    pat[:, h2 * QT + r : h2 * QT + r + P],
                                    pat[:, h2 * QT + r : h2 * QT + r + P],
                                    tri_sb[:],
                                )
                            nc.tensor.matmul(
                                acc[h2][0 : DH + 1, r:QT],
                                v_aug[pr * 2 + h2][:, kb, :],
                                pat[:, p_sl],
                                start=(kb == 0),
                                stop=(kb == nk - 1),
                            )
                        rounds_left -= 1
                        fill(rounds_left)
                    for h2 in range(2):
                        nc.vector.tensor_copy(uraw[pr][h2][:, q_sl], acc[h2][:])
                    norm_and_send(qt, pr)
                flush()
                gather(qt)
                if qt == 2:
                    units.extend(out_proj_units(0))
            flush()
            for u in out_proj_units(3):
                u()
            if dbg_d is not None:
                for pr in range(2):
                    for h2 in range(2):
                        row = (pr * 2 + h2) * (DH + 1)
                        nc.sync.dma_start(
                            dbg_d[row : row + DH + 1, :], uraw[pr][h2][:]
                        )

    nc.compile()
    return nc


def _get_nc():
    if "nc" not in _CACHE:
        _CACHE["nc"] = _build()
    return _CACHE["nc"]


def _tri():
    k = np.arange(P)[:, None]
    q = np.arange(P)[None, :]
    return (q >= k).astype(np.float32)


def _ensure_ntff_hook():
    """Register the axon NTFF profile hook (missing antenv.axon_hooks shim)."""
    import types

    try:
        from antenv.axon_hooks import get_axon_ntff_profile_hook  # noqa: F401

        return
    except ImportError:
        pass
    import antenv

    if "/root/.axon_site" not in sys.path:
        sys.path.insert(0, "/root/.axon_site")
    from trn_agent_boot.trn_boot import _ntff_profile_via_ctypes

    hook = _ntff_profile_via_ctypes("/opt/axon/libaxon_pjrt.so")
    mod = types.ModuleType("antenv.axon_hooks")
    mod.get_axon_ntff_profile_hook = lambda: hook
    mod.set_axon_ntff_profile_hook = lambda h: None
    sys.modules["antenv.axon_hooks"] = mod
    antenv.axon_hooks = mod


def kernel(residual, W_Q, W_K, W_V, W_O):
    from concourse.bass_utils import run_bass_kernel_spmd

    if int(os.environ.get("KERNEL_TRACE", "0")):
        _ensure_ntff_hook()

    residual = np.ascontiguousarray(np.asarray(residual), np.float32)
    W_Q = np.ascontiguousarray(np.asarray(W_Q), np.float32)
    W_K = np.ascontiguousarray(np.asarray(W_K), np.float32)
    W_V = np.ascontiguousarray(np.asarray(W_V), np.float32)
    W_O = np.ascontiguousarray(np.asarray(W_O), np.float32)

    nc = _get_nc()
    tri = _tri()
    wof = np.ascontiguousarray(W_O.reshape(N_HEADS * DH, D).astype(np.float16))
    in_maps = []
    for c in range(NCORES):
        b, g = divmod(c, GRP)
        hs = slice(g * NH_CORE, (g + 1) * NH_CORE)
        in_maps.append(
            {
                "xt": np.ascontiguousarray(residual[b].T.astype(np.float16)),
                "wqt": np.ascontiguousarray(
                    W_Q[hs].transpose(2, 0, 1).reshape(D, NH_CORE * DH).astype(np.float16)
                ),
                "wkt": np.ascontiguousarray(
                    W_K[hs].transpose(2, 0, 1).reshape(D, NH_CORE * DH).astype(np.float16)
                ),
                "wvt": np.ascontiguousarray(
                    W_V[hs].transpose(2, 0, 1).reshape(D, NH_CORE * DH).astype(np.float16)
                ),
                "wof": wof,
                "msk": tri.astype(np.float16),
            }
        )

    res = run_bass_kernel_spmd(
        nc,
        in_maps,
        core_ids=list(range(NCORES)),
        trace=bool(int(os.environ.get("KERNEL_TRACE", "0"))),
        trace_cores=[0] if int(os.environ.get("KERNEL_TRACE", "0")) else None,
    )
    _CACHE["last_results"] = res

    out = np.empty((B, S, D), np.float32)
    for b in range(B):
        for g in range(GRP):
            blk = np.asarray(res.results[b * GRP + g]["out"], np.float32)
            for q in range(NQT):
                out[b, q * QT + g * P : q * QT + (g + 1) * P, :] = blk[
                    q * P : (q + 1) * P
                ]
    return out


# revision 22
# speedup vs baseline: 1.7956x; 1.1392x over previous
"""Trainium2 Bass kernel for causal multi-head attention (B=2, S=2048, D=1024, 16 heads x 64).

Sharding: 8 cores = 2 batches x 4 head-groups (tensor parallel over heads).
Each core computes attention for its 4 heads over the full sequence; the 4
cores of a batch AllGather normalized head outputs per q-quarter (4-rank
groups), and each core applies the full W_O projection to one 128-row block
of every quarter (so only the final AllGather gates a small slice of work).
Host concatenates the interleaved blocks.

Attention is flash-style with transposed scores: sT[k, q] = K Q^T (keys on
partitions). Both heads of a pair write one [128, 1024] 2-bank PSUM tile so a
single ACT exp covers them. AV uses stationary [v | 1] so PSUM row 64
accumulates the softmax denominator for free; the denominator row is
broadcast to 64 partitions by a 1-contraction fp32r matmul and reciprocated
at base partition 0 with the fast custom DVE op (nonzero-base custom DVE
silently no-ops). Head-pairs (pr) run sequentially per q-tile so attention
needs only 2 accumulator banks. QKV projection and out-projection matmul
chains are chopped into single-instruction units and interleaved between
attention rounds so the PE FIFO always has dependency-free filler during
ACT-bound stretches (keeps HAM warm, overlaps phases).
"""

import os
import sys

sys.path.insert(0, "/opt/trn_rl_repo")

import numpy as np

# ---- problem constants (hardcoded; kernel.py must be self-contained) ----
B = 2
S = 2048
D = 1024
N_HEADS = 16
DH = 64                 # head dim
NCORES = 8
NH_CORE = N_HEADS // 4  # 4 heads per core (4-way TP x 2-way batch DP)
SCALE = 1.0 / 8.0       # 1/sqrt(64)

P = 128                 # partitions
DC = D // P             # 8 contraction chunks for the projections
KC = S // P             # 16 key chunks
QT = 512                # q tile width (free dim) per quarter
NQT = S // QT           # 4 q tiles
GRP = 4                 # cores per batch group

_CACHE = {}


def _build():
    import concourse.bass as bass
    import concourse.tile as tile
    from concourse import bacc, mybir

    f32 = mybir.dt.float32
    f32r = mybir.dt.float32r
    F16 = mybir.dt.float16

    nc = bacc.Bacc(
        "TRN2",
        target_bir_lowering=False,
        debug=False,
        enable_asserts=False,
        num_devices=NCORES,
    )

    xt_d = nc.dram_tensor("xt", [D, S], F16, kind="ExternalInput").ap()
    wqt_d = nc.dram_tensor("wqt", [D, NH_CORE * DH], F16, kind="ExternalInput").ap()
    wkt_d = nc.dram_tensor("wkt", [D, NH_CORE * DH], F16, kind="ExternalInput").ap()
    wvt_d = nc.dram_tensor("wvt", [D, NH_CORE * DH], F16, kind="ExternalInput").ap()
    wof_d = nc.dram_tensor("wof", [N_HEADS * DH, D], F16, kind="ExternalInput").ap()
    msk_d = nc.dram_tensor("msk", [P, P], F16, kind="ExternalInput").ap()
    # 4 blocks of 128 rows: block q = rows [q*512 + g*128 .. +128] of this
    # core's batch output (g = group rank)
    out_d = nc.dram_tensor("out", [NQT * P, D], F16, kind="ExternalOutput").ap()
    dbg_d = None
    if int(os.environ.get("KERNEL_DEBUG", "0")):
        dbg_d = nc.dram_tensor("dbg", [4 * (DH + 1), S], f32, kind="ExternalOutput").ap()
    exp_pair = int(os.environ.get("KERNEL_EXPPAIR", "1"))

    Exp = mybir.ActivationFunctionType.Exp

    with tile.TileContext(nc) as tc:
        with (
            tc.tile_pool(name="const", bufs=1) as const,
            tc.tile_pool(name="work", bufs=2) as work,
            tc.tile_pool(name="ps", bufs=1, space="PSUM") as ps_pool,
            tc.tile_pool(name="dram", bufs=1, space="DRAM") as dram,
        ):
            # ---------------- input DMAs ----------------
            wq_sb = const.tile([P, DC, NH_CORE * DH], F16)
            nc.sync.dma_start(wq_sb[:], wqt_d.rearrange("(c p) n -> p c n", p=P))
            wk_sb = const.tile([P, DC, NH_CORE * DH], F16)
            nc.sync.dma_start(wk_sb[:], wkt_d.rearrange("(c p) n -> p c n", p=P))

            # residual^T: first quarter per d-chunk (small, on gpsimd queue so
            # issue overlaps the sync-engine weight DMAs), rest in fat slices
            xt_eng = nc.gpsimd if int(os.environ.get("KERNEL_XTGP", "1")) else nc.sync
            xt_sb = const.tile([P, DC, S], F16)
            xt_r = xt_d.rearrange("(c p) s -> p c s", p=P)
            for dc in range(DC):
                xt_eng.dma_start(xt_sb[:, dc, 0:QT], xt_r[:, dc, 0:QT])
            tri_sb = const.tile([P, P], F16)
            nc.sync.dma_start(tri_sb[:], msk_d)
            wv_sb = const.tile([P, DC, NH_CORE * DH], F16)
            nc.sync.dma_start(wv_sb[:], wvt_d.rearrange("(c p) n -> p c n", p=P))
            for dc in range(DC):
                xt_eng.dma_start(xt_sb[:, dc, QT:S], xt_r[:, dc, QT:S])

            # ---------------- SBUF state ----------------
            qT = [const.tile([P, S], F16, name=f"qT{i}") for i in range(2)]
            kT = [const.tile([P, S], F16, name=f"kT{i}") for i in range(2)]
            v_aug = [const.tile([P, KC, DH + 1], F16, name=f"vaug{h}") for h in range(NH_CORE)]
            ones_f32 = const.tile([P, DH], f32)
            nc.vector.memset(ones_f32[:], 1.0)
            ones_f16 = const.tile([DH + 1, DH], F16)
            nc.vector.memset(ones_f16[:], 1.0)
            for h in range(NH_CORE):
                nc.scalar.copy(v_aug[h][:, :, DH : DH + 1], ones_f32[:, 0:KC, None])

            # unnormalized attn_outT per (pair, head): [64, S]
            uraw = [
                [const.tile([DH, S], f32, name=f"uraw{pr}{h2}") for h2 in range(2)]
                for pr in range(2)
            ]

            wo_sb = const.tile([P, DC, D], F16)

            cc_in = dram.tile([NQT * 2 * P, QT], F16)
            cc_out = dram.tile([NQT * GRP * 2 * P, QT], F16)

            pj = [0]  # alternating tag counter for the 2 shared psum banks

            def _pj_tile(shape, name):
                t = ps_pool.tile(shape, f32, name=name, tag=f"pj{pj[0] % 2}", bufs=1)
                pj[0] += 1
                return t

            # ---- filler units: single instructions emitted between rounds ----
            def qk_chain_units(nt, pr, w_sb, dst):
                st = {}
                us = []
                for dc in range(DC):
                    def mm(dc=dc, nt=nt, pr=pr, w_sb=w_sb):
                        if dc == 0:
                            st["pp"] = _pj_tile([P, QT], "pp")
                        nc.tensor.matmul(
                            st["pp"][:],
                            w_sb[:, dc, pr * P : (pr + 1) * P],
                            xt_sb[:, dc, nt * QT : (nt + 1) * QT],
                            start=(dc == 0),
                            stop=(dc == DC - 1),
                        )
                    us.append(mm)
                def cp(nt=nt, dst=dst):
                    nc.vector.tensor_copy(dst[:, nt * QT : (nt + 1) * QT], st["pp"][:])
                us.append(cp)
                return us

            def v_chain_units(pc):
                st = {}
                us = []
                for dc in range(DC):
                    def mm(dc=dc, pc=pc):
                        if dc == 0:
                            st["vp"] = _pj_tile([P, NH_CORE * DH], "vp")
                        nc.tensor.matmul(
                            st["vp"][:],
                            xt_sb[:, dc, pc * P : (pc + 1) * P],
                            wv_sb[:, dc, :],
                            start=(dc == 0),
                            stop=(dc == DC - 1),
                        )
                    us.append(mm)
                for h in range(NH_CORE):
                    def cp(h=h, pc=pc):
                        nc.vector.tensor_copy(
                            v_aug[h][:, pc, 0:DH], st["vp"][:, h * DH : (h + 1) * DH]
                        )
                    us.append(cp)
                return us

            def proj_units(nt):
                us = []
                for pr in range(2):
                    us += qk_chain_units(nt, pr, wq_sb, qT[pr])
                    us += qk_chain_units(nt, pr, wk_sb, kT[pr])
                for pc in range(4 * nt, 4 * nt + 4):
                    us += v_chain_units(pc)
                return us

            # my 128-q-column block within each quarter (dynamic by rank)
            pid = nc.partition_id()
            qoff = nc.snap(
                nc.s_assert_within(
                    (pid % GRP) * P, 0, QT - P, skip_runtime_assert=True
                )
            )
            cc_q = cc_out[:].rearrange("(t a p) q -> t p a q", p=P, a=GRP * 2)

            def out_proj_units(qt):
                st = {}
                us = []
                def dma(qt=qt):
                    st["attR"] = work.tile([P, GRP * 2, P], F16, name="attR", bufs=2)
                    nc.sync.dma_start(
                        st["attR"][:], cc_q[qt, :, :, bass.ds(qoff, P)]
                    )
                    st["osb"] = work.tile([P, D], F16, name="osb", bufs=2)
                us.append(dma)
                for dt_ in range(D // QT):
                    for c in range(GRP * 2):
                        def mm(dt_=dt_, c=c):
                            if c == 0:
                                st[f"op{dt_}"] = _pj_tile([P, QT], "op")
                            nc.tensor.matmul(
                                st[f"op{dt_}"][:],
                                st["attR"][:, c, :],
                                wo_sb[:, c, dt_ * QT : (dt_ + 1) * QT],
                                start=(c == 0),
                                stop=(c == GRP * 2 - 1),
                            )
                        us.append(mm)
                    def cp(dt_=dt_):
                        nc.vector.tensor_copy(
                            st["osb"][:, dt_ * QT : (dt_ + 1) * QT], st[f"op{dt_}"][:]
                        )
                    us.append(cp)
                def outdma(qt=qt):
                    nc.sync.dma_start(out_d[qt * P : (qt + 1) * P, :], st["osb"][:])
                us.append(outdma)
                return us

            units = []

            def fill(rounds_left):
                if not units:
                    return
                n = max(1, (len(units) + rounds_left - 1) // max(rounds_left, 1))
                for _ in range(min(n, len(units))):
                    units.pop(0)()

            def flush():
                while units:
                    units.pop(0)()

            def norm_and_send(qt, pr, dens):
                g_sl = slice(qt * QT, (qt + 1) * QT)
                for h2 in range(2):
                    u = uraw[pr][h2]
                    # broadcast f16 denominator row to 64 partitions (full-rate
                    # matmul), then fast-reciprocal at base partition 0 (the
                    # custom DVE op silently no-ops at nonzero base partitions)
                    rb_ps = _pj_tile([DH, QT], "rb")
                    nc.tensor.matmul(
                        rb_ps[:],
                        ones_f16[DH : DH + 1, :],
                        dens[h2][DH : DH + 1, :],
                        start=True,
                        stop=True,
                    )
                    rb_sb = work.tile([DH, QT], f32, name="rb_sb", bufs=2)
                    nc.vector.reciprocal_approx_fast(rb_sb[:], rb_ps[:])
                    u_n = work.tile([DH, QT], F16, name="u_n", bufs=4)
                    nc.vector.tensor_mul(u_n[:], u[0:DH, g_sl], rb_sb[:])
                    row = qt * 2 * P + pr * P + h2 * DH
                    nc.sync.dma_start(cc_in[row : row + DH, :], u_n[:])

            def gather(qt):
                nc.gpsimd.collective_compute(
                    "AllGather",
                    mybir.AluOpType.bypass,
                    replica_groups=[[0, 1, 2, 3], [4, 5, 6, 7]],
                    ins=[cc_in[qt * 2 * P : (qt + 1) * 2 * P, :].opt()],
                    outs=[
                        cc_out[qt * GRP * 2 * P : (qt + 1) * GRP * 2 * P, :].opt()
                    ],
                )

            # ---------------- main loop ----------------
            for u in proj_units(0):
                u()
            for qt in range(NQT):
                if qt == 1:
                    nc.sync.dma_start(
                        wo_sb[:], wof_d.rearrange("(c p) d -> p c d", p=P)
                    )
                # filler work for this quarter's ACT-bound attention span
                if qt + 1 < NQT:
                    units.extend(proj_units(qt + 1))
                if qt == 3:
                    units.extend(out_proj_units(1))
                    units.extend(out_proj_units(2))

                q_sl = slice(qt * QT, (qt + 1) * QT)
                nk = (qt + 1) * (QT // P)
                rounds_left = 2 * nk
                pipe = int(os.environ.get("KERNEL_PIPE", "1"))
                for pr in range(2):
                    acc = [
                        ps_pool.tile(
                            [DH + 1, QT], f32, name=f"acc{h2}", tag=f"acc{h2}", bufs=1
                        )
                        for h2 in range(2)
                    ]

                    def av_round(kb, r, pat):
                        for h2 in range(2):
                            nc.tensor.matmul(
                                acc[h2][0 : DH + 1, r:QT],
                                v_aug[pr * 2 + h2][:, kb, :],
                                pat[:, h2 * QT + r : (h2 + 1) * QT],
                                start=(kb == 0),
                                stop=(kb == nk - 1),
                            )

                    pend = None  # (kb, r, pat) awaiting its AV matmuls
                    for kb in range(nk):
                        k_sl = slice(kb * P, (kb + 1) * P)
                        ri = kb - qt * (QT // P)  # >= 0 on diagonal tiles
                        r = max(ri, 0) * P        # first valid col in this q tile
                        c_sl = slice(qt * QT + r, (qt + 1) * QT)
                        sc = ps_pool.tile([P, 2 * QT], f32, name="sc", tag="sc", bufs=2)
                        pat = work.tile([P, 2 * QT], F16, name="pat", bufs=3)
                        for h2 in range(2):
                            hb = h2 * DH
                            nc.tensor.matmul(
                                sc[:, h2 * QT + r : (h2 + 1) * QT],
                                kT[pr][hb : hb + DH, k_sl],
                                qT[pr][hb : hb + DH, c_sl],
                                start=True,
                                stop=True,
                            )
                        # one exp covers both heads (cols 512..512+r of the
                        # diagonal rounds are stale-PSUM garbage, never read)
                        if exp_pair:
                            nc.scalar.activation(
                                pat[:, r : 2 * QT], sc[:, r : 2 * QT], Exp, scale=SCALE
                            )
                        else:
                            for h2 in range(2):
                                e_sl = slice(h2 * QT + r, (h2 + 1) * QT)
                                nc.scalar.activation(
                                    pat[:, e_sl], sc[:, e_sl], Exp, scale=SCALE
                                )
                        if ri >= 0:
                            for h2 in range(2):
                                nc.vector.tensor_mul(
                                    pat[:, h2 * QT + r : h2 * QT + r + P],
                                    pat[:, h2 * QT + r : h2 * QT + r + P],
                                    tri_sb[:],
                                )
                        # software-pipeline the PE stream one round deep: this
                        # round's score MMs enter the FIFO before the previous
                        # round's exp-gated AV MMs, so the PE never sits on the
                        # ACT semaphore with an exposed LDWEIGHTS behind it
                        if pipe:
                            if pend is not None:
                                av_round(*pend)
                            pend = (kb, r, pat)
                        else:
                            av_round(kb, r, pat)
                        rounds_left -= 1
                        fill(rounds_left)
                    if pend is not None:
                        av_round(*pend)
                    dens = []
                    for h2 in range(2):
                        nc.vector.tensor_copy(uraw[pr][h2][:, q_sl], acc[h2][0:DH, :])
                        # f16 denominator stays on lane 64 (DVE can't move
                        # across partitions); the matmul streams from there
                        den = work.tile([DH + 1, QT], F16, name="den", bufs=4)
                        nc.vector.tensor_copy(
                            den[DH : DH + 1, :], acc[h2][DH : DH + 1, :]
                        )
                        dens.append(den)
                    norm_and_send(qt, pr, dens)
                flush()
                gather(qt)
                if qt == 2:
                    units.extend(out_proj_units(0))
            flush()
            for u in out_proj_units(3):
                u()
            if dbg_d is not None:
                for pr in range(2):
                    for h2 in range(2):
                        row = (pr * 2 + h2) * (DH + 1)
                        nc.sync.dma_start(
                            dbg_d[row : row + DH, :], uraw[pr][h2][:]
                        )

    nc.compile()
    return nc


def _get_nc():
    if "nc" not in _CACHE:
        _CACHE["nc"] = _build()
    return _CACHE["nc"]


def _tri():
    k = np.arange(P)[:, None]
    q = np.arange(P)[None, :]
    return (q >= k).astype(np.float32)


def _ensure_ntff_hook():
    """Register the axon NTFF profile hook (missing antenv.axon_hooks shim)."""
    import types

    try:
        from antenv.axon_hooks import get_axon_ntff_profile_hook  # noqa: F401

        return
    except ImportError:
        pass
    import antenv

    if "/root/.axon_site" not in sys.path:
        sys.path.insert(0, "/root/.axon_site")
    from trn_agent_boot.trn_boot import _ntff_profile_via_ctypes

    hook = _ntff_profile_via_ctypes("/opt/axon/libaxon_pjrt.so")
    mod = types.ModuleType("antenv.axon_hooks")
    mod.get_axon_ntff_profile_hook = lambda: hook
    mod.set_axon_ntff_profile_hook = lambda h: None
    sys.modules["antenv.axon_hooks"] = mod
    antenv.axon_hooks = mod


def kernel(residual, W_Q, W_K, W_V, W_O):
    from concourse.bass_utils import run_bass_kernel_spmd

    if int(os.environ.get("KERNEL_TRACE", "0")):
        _ensure_ntff_hook()

    residual = np.ascontiguousarray(np.asarray(residual), np.float32)
    W_Q = np.ascontiguousarray(np.asarray(W_Q), np.float32)
    W_K = np.ascontiguousarray(np.asarray(W_K), np.float32)
    W_V = np.ascontiguousarray(np.asarray(W_V), np.float32)
    W_O = np.ascontiguousarray(np.asarray(W_O), np.float32)

    nc = _get_nc()
    tri = _tri()
    wof = np.ascontiguousarray(W_O.reshape(N_HEADS * DH, D).astype(np.float16))
    in_maps = []
    for c in range(NCORES):
        b, g = divmod(c, GRP)
        hs = slice(g * NH_CORE, (g + 1) * NH_CORE)
        in_maps.append(
            {
                "xt": np.ascontiguousarray(residual[b].T.astype(np.float16)),
                "wqt": np.ascontiguousarray(
                    W_Q[hs].transpose(2, 0, 1).reshape(D, NH_CORE * DH).astype(np.float16)
                ),
                "wkt": np.ascontiguousarray(
                    W_K[hs].transpose(2, 0, 1).reshape(D, NH_CORE * DH).astype(np.float16)
                ),
                "wvt": np.ascontiguousarray(
                    W_V[hs].transpose(2, 0, 1).reshape(D, NH_CORE * DH).astype(np.float16)
                ),
                "wof": wof,
                "msk": tri.astype(np.float16),
            }
        )

    res = run_bass_kernel_spmd(
        nc,
        in_maps,
        core_ids=list(range(NCORES)),
        trace=bool(int(os.environ.get("KERNEL_TRACE", "0"))),
        trace_cores=(
            list(range(NCORES))
            if int(os.environ.get("KERNEL_TRACE_ALL", "0"))
            else [0] if int(os.environ.get("KERNEL_TRACE", "0")) else None
        ),
    )
    _CACHE["last_results"] = res

    out = np.empty((B, S, D), np.float32)
    for b in range(B):
        for g in range(GRP):
            blk = np.asarray(res.results[b * GRP + g]["out"], np.float32)
            for q in range(NQT):
                out[b, q * QT + g * P : q * QT + (g + 1) * P, :] = blk[
                    q * P : (q + 1) * P
                ]
    return out


# revision 29
# speedup vs baseline: 1.7990x; 1.0019x over previous
"""Trainium2 Bass kernel for causal multi-head attention (B=2, S=2048, D=1024, 16 heads x 64).

Sharding: 8 cores = 2 batches x 4 head-groups (tensor parallel over heads).
Each core computes attention for its 4 heads over the full sequence; the 4
cores of a batch AllGather normalized head outputs per q-quarter (4-rank
groups), and each core applies the full W_O projection to one 128-row block
of every quarter (so only the final AllGather gates a small slice of work).
Host concatenates the interleaved blocks.

Attention is flash-style with transposed scores: sT[k, q] = K Q^T (keys on
partitions). Both heads of a pair write one [128, 1024] 2-bank PSUM tile so a
single ACT exp covers them. AV uses stationary [v | 1] so PSUM row 64
accumulates the softmax denominator for free; the denominator row is
broadcast to 64 partitions by a 1-contraction fp32r matmul and reciprocated
at base partition 0 with the fast custom DVE op (nonzero-base custom DVE
silently no-ops). Head-pairs (pr) run sequentially per q-tile so attention
needs only 2 accumulator banks. QKV projection and out-projection matmul
chains are chopped into single-instruction units and interleaved between
attention rounds so the PE FIFO always has dependency-free filler during
ACT-bound stretches (keeps HAM warm, overlaps phases).
"""

import os
import sys

sys.path.insert(0, "/opt/trn_rl_repo")

import numpy as np

# ---- problem constants (hardcoded; kernel.py must be self-contained) ----
B = 2
S = 2048
D = 1024
N_HEADS = 16
DH = 64                 # head dim
NCORES = 8
NH_CORE = N_HEADS // 4  # 4 heads per core (4-way TP x 2-way batch DP)
SCALE = 1.0 / 8.0       # 1/sqrt(64)

P = 128                 # partitions
DC = D // P             # 8 contraction chunks for the projections
KC = S // P             # 16 key chunks
QT = 512                # q tile width (free dim) per quarter
NQT = S // QT           # 4 q tiles
GRP = 4                 # cores per batch group

_CACHE = {}


def _build():
    import concourse.bass as bass
    import concourse.tile as tile
    from concourse import bacc, mybir

    f32 = mybir.dt.float32
    f32r = mybir.dt.float32r
    F16 = mybir.dt.float16

    nc = bacc.Bacc(
        "TRN2",
        target_bir_lowering=False,
        debug=False,
        enable_asserts=False,
        num_devices=NCORES,
    )

    xt_d = nc.dram_tensor("xt", [D, S], F16, kind="ExternalInput").ap()
    wqt_d = nc.dram_tensor("wqt", [D, NH_CORE * DH], F16, kind="ExternalInput").ap()
    wkt_d = nc.dram_tensor("wkt", [D, NH_CORE * DH], F16, kind="ExternalInput").ap()
    wvt_d = nc.dram_tensor("wvt", [D, NH_CORE * DH], F16, kind="ExternalInput").ap()
    wof_d = nc.dram_tensor("wof", [N_HEADS * DH, D], F16, kind="ExternalInput").ap()
    msk_d = nc.dram_tensor("msk", [P, P], F16, kind="ExternalInput").ap()
    # 4 blocks of 128 rows: block q = rows [q*512 + g*128 .. +128] of this
    # core's batch output (g = group rank)
    out_d = nc.dram_tensor("out", [NQT * P, D], F16, kind="ExternalOutput").ap()
    dbg_d = None
    if int(os.environ.get("KERNEL_DEBUG", "0")):
        dbg_d = nc.dram_tensor("dbg", [4 * (DH + 1), S], f32, kind="ExternalOutput").ap()
    exp_pair = int(os.environ.get("KERNEL_EXPPAIR", "1"))

    Exp = mybir.ActivationFunctionType.Exp

    with tile.TileContext(nc) as tc:
        with (
            tc.tile_pool(name="const", bufs=1) as const,
            tc.tile_pool(name="work", bufs=2) as work,
            tc.tile_pool(name="ps", bufs=1, space="PSUM") as ps_pool,
            tc.tile_pool(name="dram", bufs=1, space="DRAM") as dram,
        ):
            # ---------------- input DMAs ----------------
            wq_sb = const.tile([P, DC, NH_CORE * DH], F16)
            nc.sync.dma_start(wq_sb[:], wqt_d.rearrange("(c p) n -> p c n", p=P))
            wk_sb = const.tile([P, DC, NH_CORE * DH], F16)
            nc.sync.dma_start(wk_sb[:], wkt_d.rearrange("(c p) n -> p c n", p=P))

            # residual^T: first quarter per d-chunk (small, on gpsimd queue so
            # issue overlaps the sync-engine weight DMAs), rest in fat slices
            xt_eng = nc.gpsimd if int(os.environ.get("KERNEL_XTGP", "1")) else nc.sync
            xt_sb = const.tile([P, DC, S], F16)
            xt_r = xt_d.rearrange("(c p) s -> p c s", p=P)
            for dc in range(DC):
                xt_eng.dma_start(xt_sb[:, dc, 0:QT], xt_r[:, dc, 0:QT])
            tri_sb = const.tile([P, P], F16)
            nc.sync.dma_start(tri_sb[:], msk_d)
            wv_sb = const.tile([P, DC, NH_CORE * DH], F16)
            nc.sync.dma_start(wv_sb[:], wvt_d.rearrange("(c p) n -> p c n", p=P))
            for dc in range(DC):
                xt_eng.dma_start(xt_sb[:, dc, QT:S], xt_r[:, dc, QT:S])

            # ---------------- SBUF state ----------------
            qT = [const.tile([P, S], F16, name=f"qT{i}") for i in range(2)]
            kT = [const.tile([P, S], F16, name=f"kT{i}") for i in range(2)]
            v_aug = [const.tile([P, KC, DH + 1], F16, name=f"vaug{h}") for h in range(NH_CORE)]
            ones_f32 = const.tile([P, DH], f32)
            nc.vector.memset(ones_f32[:], 1.0)
            ones_f16 = const.tile([DH + 1, DH], F16)
            nc.vector.memset(ones_f16[:], 1.0)
            for h in range(NH_CORE):
                nc.scalar.copy(v_aug[h][:, :, DH : DH + 1], ones_f32[:, 0:KC, None])

            # unnormalized attn_outT per (pair, head): [64, S]
            uraw = [
                [const.tile([DH, S], f32, name=f"uraw{pr}{h2}") for h2 in range(2)]
                for pr in range(2)
            ]

            wo_sb = const.tile([P, DC, D], F16)

            cc_in = dram.tile([NQT * 2 * P, QT], F16)
            cc_out = dram.tile([NQT * GRP * 2 * P, QT], F16)

            pj = [0]  # alternating tag counter for the 2 shared psum banks

            def _pj_tile(shape, name):
                t = ps_pool.tile(shape, f32, name=name, tag=f"pj{pj[0] % 2}", bufs=1)
                pj[0] += 1
                return t

            # ---- filler units: single instructions emitted between rounds ----
            def qk_chain_units(nt, pr, w_sb, dst):
                st = {}
                us = []
                for dc in range(DC):
                    def mm(dc=dc, nt=nt, pr=pr, w_sb=w_sb):
                        if dc == 0:
                            st["pp"] = _pj_tile([P, QT], "pp")
                        nc.tensor.matmul(
                            st["pp"][:],
                            w_sb[:, dc, pr * P : (pr + 1) * P],
                            xt_sb[:, dc, nt * QT : (nt + 1) * QT],
                            start=(dc == 0),
                            stop=(dc == DC - 1),
                        )
                    us.append(mm)
                def cp(nt=nt, dst=dst):
                    nc.vector.tensor_copy(dst[:, nt * QT : (nt + 1) * QT], st["pp"][:])
                us.append(cp)
                return us

            def v_chain_units(pc):
                st = {}
                us = []
                for dc in range(DC):
                    def mm(dc=dc, pc=pc):
                        if dc == 0:
                            st["vp"] = _pj_tile([P, NH_CORE * DH], "vp")
                        nc.tensor.matmul(
                            st["vp"][:],
                            xt_sb[:, dc, pc * P : (pc + 1) * P],
                            wv_sb[:, dc, :],
                            start=(dc == 0),
                            stop=(dc == DC - 1),
                        )
                    us.append(mm)
                for h in range(NH_CORE):
                    def cp(h=h, pc=pc):
                        nc.vector.tensor_copy(
                            v_aug[h][:, pc, 0:DH], st["vp"][:, h * DH : (h + 1) * DH]
                        )
                    us.append(cp)
                return us

            def proj_units(nt):
                us = []
                for pr in range(2):
                    us += qk_chain_units(nt, pr, wq_sb, qT[pr])
                    us += qk_chain_units(nt, pr, wk_sb, kT[pr])
                for pc in range(4 * nt, 4 * nt + 4):
                    us += v_chain_units(pc)
                return us

            # my 128-q-column block within each quarter (dynamic by rank)
            pid = nc.partition_id()
            qoff = nc.snap(
                nc.s_assert_within(
                    (pid % GRP) * P, 0, QT - P, skip_runtime_assert=True
                )
            )
            cc_q = cc_out[:].rearrange("(t a p) q -> t p a q", p=P, a=GRP * 2)

            def out_proj_units(qt):
                """Two phases: (a) after gather(qt, pr=0) — attR chunks 0-3
                (W_O chunks 0,2,4,6); (b) after gather(qt, pr=1) — chunks 4-7
                (W_O chunks 1,3,5,7) + evacuate + store. PSUM accumulates
                across the phase boundary."""
                st = {}
                ua, ub = [], []
                def dma_a(qt=qt):
                    st["attR"] = work.tile([P, GRP * 2, P], F16, name="attR", bufs=2)
                    nc.sync.dma_start(
                        st["attR"][:, 0:GRP, :], cc_q[qt, :, 0:GRP, bass.ds(qoff, P)]
                    )
                    st["osb"] = work.tile([P, D], F16, name="osb", bufs=2)
                ua.append(dma_a)
                for dt_ in range(D // QT):
                    for i in range(GRP):
                        def mm(dt_=dt_, i=i):
                            if i == 0:
                                st[f"op{dt_}"] = _pj_tile([P, QT], "op")
                            nc.tensor.matmul(
                                st[f"op{dt_}"][:],
                                st["attR"][:, i, :],
                                wo_sb[:, 2 * i, dt_ * QT : (dt_ + 1) * QT],
                                start=(i == 0),
                                stop=False,
                            )
                        ua.append(mm)
                def dma_b(qt=qt):
                    nc.sync.dma_start(
                        st["attR"][:, GRP : 2 * GRP, :],
                        cc_q[qt, :, GRP : 2 * GRP, bass.ds(qoff, P)],
                    )
                ub.append(dma_b)
                for dt_ in range(D // QT):
                    for i in range(GRP):
                        def mm(dt_=dt_, i=i):
                            nc.tensor.matmul(
                                st[f"op{dt_}"][:],
                                st["attR"][:, GRP + i, :],
                                wo_sb[:, 2 * i + 1, dt_ * QT : (dt_ + 1) * QT],
                                start=False,
                                stop=(i == GRP - 1),
                            )
                        ub.append(mm)
                    def cp(dt_=dt_):
                        nc.vector.tensor_copy(
                            st["osb"][:, dt_ * QT : (dt_ + 1) * QT], st[f"op{dt_}"][:]
                        )
                    ub.append(cp)
                def outdma(qt=qt):
                    nc.sync.dma_start(out_d[qt * P : (qt + 1) * P, :], st["osb"][:])
                ub.append(outdma)
                return ua, ub

            units = []

            def fill(rounds_left):
                if not units:
                    return
                n = max(1, (len(units) + rounds_left - 1) // max(rounds_left, 1))
                for _ in range(min(n, len(units))):
                    units.pop(0)()

            def flush():
                while units:
                    units.pop(0)()

            def norm_and_send(qt, pr, dens, rb_tag=None):
                g_sl = slice(qt * QT, (qt + 1) * QT)
                for h2 in range(2):
                    u = uraw[pr][h2]
                    # broadcast f16 denominator row to 64 partitions (full-rate
                    # matmul), then fast-reciprocal at base partition 0 (the
                    # custom DVE op silently no-ops at nonzero base partitions)
                    if rb_tag is None:
                        rb_ps = _pj_tile([DH, QT], "rb")
                    else:
                        rb_ps = ps_pool.tile(
                            [DH, QT], f32, name="rb", tag=rb_tag, bufs=2
                        )
                    nc.tensor.matmul(
                        rb_ps[:],
                        ones_f16[DH : DH + 1, :],
                        dens[h2][DH : DH + 1, :],
                        start=True,
                        stop=True,
                    )
                    rb_sb = work.tile([DH, QT], f32, name="rb_sb", bufs=2)
                    nc.vector.reciprocal_approx_fast(rb_sb[:], rb_ps[:])
                    u_n = work.tile([DH, QT], F16, name="u_n", bufs=4)
                    nc.vector.tensor_mul(u_n[:], u[0:DH, g_sl], rb_sb[:])
                    row = qt * 2 * P + pr * P + h2 * DH
                    nc.sync.dma_start(cc_in[row : row + DH, :], u_n[:])

            def gather(qt, pr):
                ri = qt * 2 * P + pr * P
                ro = qt * GRP * 2 * P + pr * GRP * P
                nc.gpsimd.collective_compute(
                    "AllGather",
                    mybir.AluOpType.bypass,
                    replica_groups=[[0, 1, 2, 3], [4, 5, 6, 7]],
                    ins=[cc_in[ri : ri + P, :].opt()],
                    outs=[cc_out[ro : ro + GRP * P, :].opt()],
                )

            # ---------------- main loop ----------------
            op3b = [None]
            for u in proj_units(0):
                u()
            for qt in range(NQT):
                if qt == 1:
                    nc.sync.dma_start(
                        wo_sb[:], wof_d.rearrange("(c p) d -> p c d", p=P)
                    )
                # filler work for this quarter's ACT-bound attention span
                if qt + 1 < NQT:
                    units.extend(proj_units(qt + 1))
                if qt == 2:
                    a0, b0 = out_proj_units(0)
                    units.extend(a0)
                    units.extend(b0)
                if qt == 3:
                    for q_ in (1, 2):
                        a_, b_ = out_proj_units(q_)
                        units.extend(a_)
                        units.extend(b_)

                q_sl = slice(qt * QT, (qt + 1) * QT)
                nk = (qt + 1) * (QT // P)
                rounds_left = 2 * nk
                pipe = int(os.environ.get("KERNEL_PIPE", "1"))
                for pr in range(2):
                    acc = [
                        ps_pool.tile(
                            [DH + 1, QT], f32, name=f"acc{h2}", tag=f"acc{h2}", bufs=1
                        )
                        for h2 in range(2)
                    ]

                    def av_round(kb, r, pat):
                        for h2 in range(2):
                            nc.tensor.matmul(
                                acc[h2][0 : DH + 1, r:QT],
                                v_aug[pr * 2 + h2][:, kb, :],
                                pat[:, h2 * QT + r : (h2 + 1) * QT],
                                start=(kb == 0),
                                stop=(kb == nk - 1),
                            )

                    pend = None  # (kb, r, pat) awaiting its AV matmuls
                    for kb in range(nk):
                        k_sl = slice(kb * P, (kb + 1) * P)
                        ri = kb - qt * (QT // P)  # >= 0 on diagonal tiles
                        r = max(ri, 0) * P        # first valid col in this q tile
                        c_sl = slice(qt * QT + r, (qt + 1) * QT)
                        sc = ps_pool.tile([P, 2 * QT], f32, name="sc", tag="sc", bufs=2)
                        pat = work.tile([P, 2 * QT], F16, name="pat", bufs=3)
                        for h2 in range(2):
                            hb = h2 * DH
                            nc.tensor.matmul(
                                sc[:, h2 * QT + r : (h2 + 1) * QT],
                                kT[pr][hb : hb + DH, k_sl],
                                qT[pr][hb : hb + DH, c_sl],
                                start=True,
                                stop=True,
                            )
                        # one exp covers both heads (cols 512..512+r of the
                        # diagonal rounds are stale-PSUM garbage, never read)
                        if exp_pair:
                            nc.scalar.activation(
                                pat[:, r : 2 * QT], sc[:, r : 2 * QT], Exp, scale=SCALE
                            )
                        else:
                            for h2 in range(2):
                                e_sl = slice(h2 * QT + r, (h2 + 1) * QT)
                                nc.scalar.activation(
                                    pat[:, e_sl], sc[:, e_sl], Exp, scale=SCALE
                                )
                        if ri >= 0:
                            for h2 in range(2):
                                nc.vector.tensor_mul(
                                    pat[:, h2 * QT + r : h2 * QT + r + P],
                                    pat[:, h2 * QT + r : h2 * QT + r + P],
                                    tri_sb[:],
                                )
                        # software-pipeline the PE stream one round deep: this
                        # round's score MMs enter the FIFO before the previous
                        # round's exp-gated AV MMs, so the PE never sits on the
                        # ACT semaphore with an exposed LDWEIGHTS behind it
                        if pipe:
                            if pend is not None:
                                av_round(*pend)
                            pend = (kb, r, pat)
                        else:
                            av_round(kb, r, pat)
                        rounds_left -= 1
                        fill(rounds_left)
                    if pend is not None:
                        av_round(*pend)
                    dens = []
                    for h2 in range(2):
                        nc.vector.tensor_copy(uraw[pr][h2][:, q_sl], acc[h2][0:DH, :])
                        # f16 denominator stays on lane 64 (DVE can't move
                        # across partitions); the matmul streams from there
                        den = work.tile([DH + 1, QT], F16, name="den", bufs=4)
                        nc.vector.tensor_copy(
                            den[DH : DH + 1, :], acc[h2][DH : DH + 1, :]
                        )
                        dens.append(den)
                    # the very last norm routes its broadcast through the idle
                    # score banks: the pj banks are held by out_proj(3) phase-a
                    # at that point and waiting on them would deadlock the
                    # release chain (rb -> gather(3,1) -> phase-b -> release)
                    last = qt == NQT - 1 and pr == 1
                    norm_and_send(qt, pr, dens, rb_tag="sc" if last else None)
                    gather(qt, pr)
                    if qt == NQT - 1 and pr == 0:
                        a3, op3b[0] = out_proj_units(3)
                        units.extend(a3)
            flush()
            for u in op3b[0]:
                u()
            if dbg_d is not None:
                for pr in range(2):
                    for h2 in range(2):
                        row = (pr * 2 + h2) * (DH + 1)
                        nc.sync.dma_start(
                            dbg_d[row : row + DH, :], uraw[pr][h2][:]
                        )

    nc.compile()
    return nc


def _get_nc():
    if "nc" not in _CACHE:
        _CACHE["nc"] = _build()
    return _CACHE["nc"]


def _tri():
    k = np.arange(P)[:, None]
    q = np.arange(P)[None, :]
    return (q >= k).astype(np.float32)


def _ensure_ntff_hook():
    """Register the axon NTFF profile hook (missing antenv.axon_hooks shim)."""
    import types

    try:
        from antenv.axon_hooks import get_axon_ntff_profile_hook  # noqa: F401

        return
    except ImportError:
        pass
    import antenv

    if "/root/.axon_site" not in sys.path:
        sys.path.insert(0, "/root/.axon_site")
    from trn_agent_boot.trn_boot import _ntff_profile_via_ctypes

    hook = _ntff_profile_via_ctypes("/opt/axon/libaxon_pjrt.so")
    mod = types.ModuleType("antenv.axon_hooks")
    mod.get_axon_ntff_profile_hook = lambda: hook
    mod.set_axon_ntff_profile_hook = lambda h: None
    sys.modules["antenv.axon_hooks"] = mod
    antenv.axon_hooks = mod


def kernel(residual, W_Q, W_K, W_V, W_O):
    from concourse.bass_utils import run_bass_kernel_spmd

    if int(os.environ.get("KERNEL_TRACE", "0")):
        _ensure_ntff_hook()

    residual = np.ascontiguousarray(np.asarray(residual), np.float32)
    W_Q = np.ascontiguousarray(np.asarray(W_Q), np.float32)
    W_K = np.ascontiguousarray(np.asarray(W_K), np.float32)
    W_V = np.ascontiguousarray(np.asarray(W_V), np.float32)
    W_O = np.ascontiguousarray(np.asarray(W_O), np.float32)

    nc = _get_nc()
    tri = _tri()
    wof = np.ascontiguousarray(W_O.reshape(N_HEADS * DH, D).astype(np.float16))
    in_maps = []
    for c in range(NCORES):
        b, g = divmod(c, GRP)
        hs = slice(g * NH_CORE, (g + 1) * NH_CORE)
        in_maps.append(
            {
                "xt": np.ascontiguousarray(residual[b].T.astype(np.float16)),
                "wqt": np.ascontiguousarray(
                    W_Q[hs].transpose(2, 0, 1).reshape(D, NH_CORE * DH).astype(np.float16)
                ),
                "wkt": np.ascontiguousarray(
                    W_K[hs].transpose(2, 0, 1).reshape(D, NH_CORE * DH).astype(np.float16)
                ),
                "wvt": np.ascontiguousarray(
                    W_V[hs].transpose(2, 0, 1).reshape(D, NH_CORE * DH).astype(np.float16)
                ),
                "wof": wof,
                "msk": tri.astype(np.float16),
            }
        )

    res = run_bass_kernel_spmd(
        nc,
        in_maps,
        core_ids=list(range(NCORES)),
        trace=bool(int(os.environ.get("KERNEL_TRACE", "0"))),
        trace_cores=(
            list(range(NCORES))
            if int(os.environ.get("KERNEL_TRACE_ALL", "0"))
            else [0] if int(os.environ.get("KERNEL_TRACE", "0")) else None
        ),
    )
    _CACHE["last_results"] = res

    out = np.empty((B, S, D), np.float32)
    for b in range(B):
        for g in range(GRP):
            blk = np.asarray(res.results[b * GRP + g]["out"], np.float32)
            for q in range(NQT):
                out[b, q * QT + g * P : q * QT + (g + 1) * P, :] = blk[
                    q * P : (q + 1) * P
                ]
    return out
